# revision 18
# baseline (speedup 1.0000x reference)
"""Trainium2 Bass kernel for nn_Attention_45749991637079.

Reference computation (per batch b, C=192 channels, 128x128 image, 8 heads):
    qkv  = w_qkv @ x                       (1x1 conv; k-branch unused)
    q,v  = depthwise 3x3 (SAME) of the q/v channel blocks
    qd   = q[:, ::2, ::2]                  (64x64 downsample)
    attn = softmax(l2norm-rows(qd_h) gram * temp)   per head (24x24)
    out  = w_proj @ (attn @ v)             == (w_proj @ blockdiag(attn)) @ v

Sharding: data-parallel over batch; one batch per NeuronCore (8 cores).

Device algorithm per core (half-image phases, long homogeneous PE bursts):
  q-pw (half0, half1): pointwise conv -> padded SBUF buffers.
  q-taps: depthwise via 6 concurrent 32x32 diag tile-matmuls per tap
    (tile_position packing: chunk0 channels on the 4 diagonal tiles,
    chunk1 at (64,0)/(96,32)), PSUM-accumulated -> qd; row sumsq on ACT.
  qd -> qdT via DMA-xbar transposes (off the PE), gram accumulated in a
  single PSUM bank over 64 matmuls; softmax/Wf on DVE/ACT overlap v phase.
  v-pw + v-taps (same packing, full-res), final out = WfT @ v_dw pipelined
  with fp16 output DMA.

Channel layout: chunk0 = ch 0:128 on partitions 0:128; chunk1 = ch 128:192
  held at partitions 64:128 for x / pw weights / padded pw buffers (so tap
  tiles read row-groups 2-3), and at partitions 0:64 for tap outputs and
  everything downstream (qd, v_dw, gram, Wf), matching matmul base-partition
  pairing rules throughout.
"""

import numpy as np

C = 192
H = W = 128
HW = H * W
HEADS = 8
CHD = 24
P0, P1 = 128, 64
PBW = 130                 # padded row width (1 + 128 + 1)
PBR = 66                  # padded rows per half buffer
PBSZ = PBR * PBW          # padded half cols per chunk
HALF = HW // 2            # 8192 px
QHALF = 2048              # qd px per half
SUB = 512
TAPS = [(di, dj) for di in range(3) for dj in range(3)]

_BUILT = {}


def _build(iters=1):
    import concourse.mybir as mybir
    import concourse.tile as tile
    from concourse import bacc

    f32 = mybir.dt.float32
    f16 = mybir.dt.float16
    Alu = mybir.AluOpType
    Act = mybir.ActivationFunctionType
    Ax = mybir.AxisListType

    nc = bacc.Bacc(
        "TRN2", target_bir_lowering=False, debug=False,
        enable_asserts=False, num_devices=8,
    )

    xb = nc.dram_tensor("xb", (C, HW), f16, kind="ExternalInput").ap()
    wq = nc.dram_tensor("wq", (P0, 384), f16, kind="ExternalInput").ap()
    wv = nc.dram_tensor("wv", (P0, 384), f16, kind="ExternalInput").ap()
    wp = nc.dram_tensor("wp", (P0, 384), f32, kind="ExternalInput").ap()
    dqp = nc.dram_tensor("dqp", (P0, 576), f16, kind="ExternalInput").ap()
    dvp = nc.dram_tensor("dvp", (P0, 576), f16, kind="ExternalInput").ap()
    tq = nc.dram_tensor("tq", (C, 1), f32, kind="ExternalInput").ap()
    out = nc.dram_tensor("out", (C, HW), f16, kind="ExternalOutput").ap()
    import os
    _dbg = os.environ.get("KDBG") == "1"
    if _dbg:
        dqd = nc.dram_tensor("dqd", (P0, 8192), f16, kind="ExternalOutput").ap()
        dgram = nc.dram_tensor("dgram", (CHD, C), f32, kind="ExternalOutput").ap()
        datt = nc.dram_tensor("datt", (CHD, C), f32, kind="ExternalOutput").ap()
        dwf = nc.dram_tensor("dwf", (P0, 384), f16, kind="ExternalOutput").ap()

    import contextlib

    with tile.TileContext(nc) as tc:
      with (tc.For_i(0, iters, 1) if iters > 1 else contextlib.nullcontext()):
        with (
            tc.tile_pool(name="const", bufs=1) as cp,
            tc.tile_pool(name="pad", bufs=1) as padp,
            tc.tile_pool(name="vdw", bufs=2) as vdwp,
            tc.tile_pool(name="ost", bufs=3) as ostp,
            tc.tile_pool(name="ps", bufs=2, space="PSUM") as ps,
            tc.tile_pool(name="ps1", bufs=1, space="PSUM") as ps1,
        ):
            # ---- constants ----
            wq_sb = cp.tile([P0, 384], f16)
            wv_sb = cp.tile([P0, 384], f16)
            wp_sb = cp.tile([P0, 384], f32)
            dqp_sb = cp.tile([P0, 576], f16)
            dvp_sb = cp.tile([P0, 576], f16)
            tq_sb = cp.tile([P0, 2], f32)
            x_sb = cp.tile([P0, 2 * HW], f16)
            qd_sb = cp.tile([P0, 8192], f16)   # chunk0 0:4096 | chunk1 [0:64] 4096:8192
            qdT = cp.tile([P0, 32 * C], f16)   # 32 px-blocks x (128|64) ch cols
            g0a = cp.tile([P0, C], f32)
            g1a = cp.tile([P1, C], f32)
            srow = cp.tile([P0, C], f32)
            att = cp.tile([CHD, C], f32)
            sm8 = cp.tile([CHD, 4 * HEADS], f32)
            rn = cp.tile([P0, 2], f32)
            ssq = cp.tile([P0, 32], f32)       # chunk0 cols 0:16, chunk1 16:32
            scr = cp.tile([P0, SUB], f32)
            A0 = cp.tile([P0, C], f32)
            A1 = cp.tile([P1, C], f32)
            wf_sb = cp.tile([P0, 384], f16)

            nc.sync.dma_start(out=wq_sb[:, 0:192], in_=wq[:, 0:192])
            nc.sync.dma_start(out=wq_sb[64:128, 192:384], in_=wq[64:128, 192:384])
            nc.sync.dma_start(out=wv_sb[:, 0:192], in_=wv[:, 0:192])
            nc.sync.dma_start(out=wv_sb[64:128, 192:384], in_=wv[64:128, 192:384])
            nc.sync.dma_start(out=wp_sb[:, 0:192], in_=wp[:, 0:192])
            nc.sync.dma_start(out=wp_sb[0:P1, 192:384], in_=wp[0:P1, 192:384])
            nc.sync.dma_start(out=dqp_sb[:], in_=dqp[:])
            nc.sync.dma_start(out=dvp_sb[:], in_=dvp[:])
            nc.sync.dma_start(out=tq_sb[:, 0:1], in_=tq[0:P0, :])
            nc.sync.dma_start(out=tq_sb[0:P1, 1:2], in_=tq[P0:C, :])

            def dma_x(r0, r1):
                nc.sync.dma_start(out=x_sb[:, r0 * W:r1 * W],
                                  in_=xb[0:P0, r0 * W:r1 * W])
                nc.sync.dma_start(out=x_sb[64:128, HW + r0 * W:HW + r1 * W],
                                  in_=xb[P0:C, r0 * W:r1 * W])

            def pw_half(w_sb, pb, img_lo, img_hi, dst_of):
                """Pointwise conv rows [img_lo, img_hi) into padded buffer pb
                (pb row = img_row + dst_of). Zero pads emitted by caller."""
                pbv0 = pb[:, 0:PBSZ].rearrange("p (r c) -> p r c", c=PBW)
                pbv1 = pb[64:128, PBSZ:2 * PBSZ].rearrange(
                    "p (r c) -> p r c", c=PBW)
                r = img_lo
                while r < img_hi:
                    nr = min(4, img_hi - r)
                    if (img_hi - r) % 4 != 0:
                        nr = (img_hi - r) % 4
                    ncol = nr * W
                    px0 = r * W
                    ppw0 = ps.tile([P0, SUB], f32, tag="pw0")
                    ppw1 = ps.tile([P0, SUB], f32, tag="pw1")
                    nc.tensor.matmul(ppw0[:, 0:ncol], w_sb[:, 0:128],
                                     x_sb[:, px0:px0 + ncol],
                                     start=True, stop=False)
                    nc.tensor.matmul(ppw0[:, 0:ncol], w_sb[64:128, 192:320],
                                     x_sb[64:128, HW + px0:HW + px0 + ncol],
                                     start=False, stop=True)
                    nc.tensor.matmul(ppw1[64:128, 0:ncol], w_sb[:, 128:192],
                                     x_sb[:, px0:px0 + ncol],
                                     start=True, stop=False)
                    nc.tensor.matmul(ppw1[64:128, 0:ncol], w_sb[64:128, 320:384],
                                     x_sb[64:128, HW + px0:HW + px0 + ncol],
                                     start=False, stop=True)
                    dr = r + dst_of
                    nc.vector.tensor_copy(
                        pbv0[:, dr:dr + nr, 1:129],
                        ppw0[:, 0:ncol].rearrange("p (r c) -> p r c", c=W))
                    nc.scalar.copy(
                        pbv1[:, dr:dr + nr, 1:129],
                        ppw1[64:128, 0:ncol].rearrange("p (r c) -> p r c", c=W))
                    r += nr

            def pad_zeros(pb, zero_top, zero_bot):
                pbv0 = pb[:, 0:PBSZ].rearrange("p (r c) -> p r c", c=PBW)
                pbv1 = pb[64:128, PBSZ:2 * PBSZ].rearrange(
                    "p (r c) -> p r c", c=PBW)
                for v in (pbv0, pbv1):
                    nc.gpsimd.memset(v[:, :, 0:1], 0.0)
                    nc.gpsimd.memset(v[:, :, 129:130], 0.0)
                    if zero_top:
                        nc.gpsimd.memset(v[:, 0, :], 0.0)
                    if zero_bot:
                        nc.gpsimd.memset(v[:, PBR - 1, :], 0.0)

            def taps(pb, d_sb, blk_rows, nblk, stride, o_c, sink):
                """Depthwise taps for one half. Per block: 6 packed 32x32
                tile-matmuls per tap via tile_position (chunk0 on the 4
                diagonal tiles, chunk1 at (64,0)/(96,32)), 9 taps
                PSUM-accumulated. sink(blk, pt0, pt1) consumes psum tiles."""
                pbv0 = pb[:, 0:PBSZ].rearrange("p (r c) -> p r c", c=PBW)
                pbv1 = pb[64:128, PBSZ:2 * PBSZ].rearrange(
                    "p (r c) -> p r c", c=PBW)
                for blk in range(nblk):
                    pt0 = ps.tile([P0, SUB], f32, tag="tap0")
                    pt1 = ps1.tile([P0, SUB], f32, tag="tap1")
                    o0 = pt0[:].rearrange("p (r c) -> p r c", c=o_c)
                    o1 = pt1[0:64].rearrange("p (r c) -> p r c", c=o_c)
                    nrow = SUB // o_c
                    r0 = blk * blk_rows
                    for t, (di, dj) in enumerate(TAPS):
                        for g in range(4):
                            gp = slice(32 * g, 32 * g + 32)
                            nc.tensor.matmul(
                                o0[gp],
                                d_sb[gp, 32 * t:32 * t + 32],
                                pbv0[gp,
                                     r0 + di:r0 + di + nrow * stride:stride,
                                     dj:dj + W:stride],
                                start=(t == 0),
                                stop=(t == 8),
                                tile_position=(32 * g, 32 * g),
                                skip_group_check=True)
                        for j in range(2):
                            nc.tensor.matmul(
                                o1[32 * j:32 * j + 32],
                                d_sb[64 + 32 * j:96 + 32 * j,
                                     288 + 32 * t:288 + 32 * t + 32],
                                pbv1[32 * j:32 * j + 32,
                                     r0 + di:r0 + di + nrow * stride:stride,
                                     dj:dj + W:stride],
                                start=(t == 0),
                                stop=(t == 8),
                                tile_position=(64 + 32 * j, 32 * j),
                                skip_group_check=True)
                    sink(blk, pt0, pt1)

            # ================= q phase =================
            pb_q = {}
            for h in range(2):
                pb = padp.tile([P0, 2 * PBSZ], f16, tag="pad")
                pb_q[h] = pb
                pad_zeros(pb, zero_top=(h == 0), zero_bot=False)
                if h == 0:
                    for cc in range(4):
                        dma_x(16 * cc, 16 * cc + 16)
                    pw_half(wq_sb, pb, 0, 64, 1)
                else:
                    for cc in range(4, 8):
                        dma_x(16 * cc, 16 * cc + 16)
                    pw_half(wq_sb, pb, 63, 128, -63)

                def q_sink(blk, pt0, pt1, h=h):
                    base = h * QHALF + blk * SUB
                    col = h * 4 + blk
                    nc.scalar.activation(scr[:], pt0[:], Act.Square,
                                         accum_out=ssq[:, col:col + 1])
                    nc.scalar.activation(scr[0:64], pt1[0:64], Act.Square,
                                         accum_out=ssq[0:64, 16 + col:17 + col])
                    nc.vector.tensor_copy(qd_sb[:, base:base + SUB], pt0[:])
                    nc.vector.tensor_copy(
                        qd_sb[0:64, 4096 + base:4096 + base + SUB], pt1[0:64])
                    for k in range(4):
                        kk = (base + k * 128) // 128
                        eng = nc.scalar
                        eng.dma_start_transpose(
                            out=qdT[:, kk * C:kk * C + 128],
                            in_=qd_sb[:, kk * 128:kk * 128 + 128])
                        eng.dma_start_transpose(
                            out=qdT[:, kk * C + 128:kk * C + 192],
                            in_=qd_sb[0:64, 4096 + kk * 128:4096 + kk * 128 + 128])

                taps(pb, dqp_sb, 16, 4, 2, 64, q_sink)

            # ================= v phase h0 =================
            pb = padp.tile([P0, 2 * PBSZ], f16, tag="pad")
            pad_zeros(pb, zero_top=True, zero_bot=False)
            pw_half(wv_sb, pb, 0, 65, 1)
            vq01 = [vdwp.tile([P0, 2 * 4096], f16, tag="vdw",
                              name=f"vdw0_{i}") for i in range(2)]

            def v_sink0(blk, pt0, pt1):
                qt = vq01[blk // 8]
                o = (blk % 8) * SUB
                nc.vector.tensor_copy(qt[:, o:o + SUB], pt0[:])
                nc.scalar.copy(qt[0:64, 4096 + o:4096 + o + SUB], pt1[0:64])

            taps(pb, dvp_sb, 4, 16, 1, 128, v_sink0)

            # ---- gram: G = qdT^T @ qdT accumulated in one PSUM bank ----
            G = ps1.tile([P0, 384], f32, tag="gacc")
            for kk in range(32):
                b = kk * C
                nc.tensor.matmul(G[:, 0:192], qdT[:, b:b + 128],
                                 qdT[:, b:b + 192],
                                 start=(kk == 0), stop=(kk == 31),
                                 skip_group_check=True)
            for kk in range(32):
                b = kk * C
                nc.tensor.matmul(G[0:64, 192:384], qdT[:, b + 128:b + 192],
                                 qdT[:, b:b + 192],
                                 start=False, stop=(kk == 31),
                                 skip_group_check=True)
            nc.vector.tensor_copy(g0a[:], G[:, 0:192])
            nc.scalar.copy(g1a[:], G[0:64, 192:384])
            if _dbg:
                nc.sync.dma_start(out=dqd[:], in_=qd_sb[:])

            # ---- row scales: rn = sqrt(temp) / ||qd_row|| (Newton-refined) ----
            nc.vector.tensor_reduce(ssq[:, 0:1], ssq[:, 0:8], Ax.X, Alu.add)
            nc.vector.tensor_reduce(ssq[0:64, 16:17], ssq[0:64, 16:24],
                                    Ax.X, Alu.add)
            for ss_ap, rn_ap, tq_ap in (
                (ssq[:, 0:1], rn[:, 0:1], tq_sb[:, 0:1]),
                (ssq[0:64, 16:17], rn[0:64, 1:2], tq_sb[0:64, 1:2]),
            ):
                y = scr[0:ss_ap.shape[0], 0:1]
                yr = scr[0:ss_ap.shape[0], 1:2]
                nc.scalar.activation(y, ss_ap, Act.Sqrt)
                nc.vector.reciprocal(yr, y)
                nc.vector.tensor_tensor(yr, yr, ss_ap, Alu.mult)
                nc.vector.tensor_tensor(y, y, yr, Alu.add)
                nc.vector.tensor_scalar_mul(y, y, 0.5)
                nc.vector.reciprocal(rn_ap, y)
                nc.vector.tensor_tensor(rn_ap, rn_ap, tq_ap, Alu.mult)

            nc.sync.dma_start(out=srow[0:1, 0:P0], in_=rn[:, 0:1])
            nc.sync.dma_start(out=srow[0:1, P0:C], in_=rn[0:P1, 1:2])
            nc.gpsimd.partition_broadcast(srow[:], srow[0:1, :])
            nc.vector.tensor_scalar_mul(g0a[:], g0a[:], rn[:, 0:1])
            nc.vector.tensor_scalar_mul(g1a[:], g1a[:], rn[0:P1, 1:2])
            nc.vector.tensor_tensor(g0a[:], g0a[:], srow[:], Alu.mult)
            nc.vector.tensor_tensor(g1a[:], g1a[:], srow[0:P1, :], Alu.mult)

            # ---- per-head diag blocks -> compact (24, 8*24) via DMA ----
            for hh in range(HEADS):
                c0 = hh * CHD
                cs = slice(c0, c0 + CHD)
                dst = att[:, cs]
                if c0 + CHD <= P0:
                    nc.sync.dma_start(out=dst, in_=g0a[cs, cs])
                elif c0 >= P0:
                    nc.sync.dma_start(out=dst, in_=g1a[c0 - P0:c0 - P0 + CHD, cs])
                else:
                    n0 = P0 - c0
                    nc.sync.dma_start(out=att[0:n0, cs], in_=g0a[c0:P0, cs])
                    nc.sync.dma_start(out=att[n0:CHD, cs],
                                      in_=g1a[0:CHD - n0, cs])
            if _dbg:
                nc.sync.dma_start(out=dgram, in_=att[:])

            # ---- softmax over d within each head block ----
            attv = att[:].rearrange("p (h c) -> p h c", c=CHD)
            mx = sm8[:, 0:HEADS]
            nc.vector.tensor_reduce(mx, attv, Ax.X, Alu.max)
            nc.vector.tensor_tensor(attv, attv,
                                    mx.unsqueeze(2).broadcast_to((CHD, HEADS, CHD)),
                                    Alu.subtract)
            nc.scalar.activation(att[:], att[:], Act.Exp)
            sm = sm8[:, HEADS:2 * HEADS]
            nc.vector.tensor_reduce(sm, attv, Ax.X, Alu.add)
            rs = sm8[:, 2 * HEADS:3 * HEADS]
            nc.vector.reciprocal(rs, sm)
            nc.vector.tensor_tensor(attv, attv,
                                    rs.unsqueeze(2).broadcast_to((CHD, HEADS, CHD)),
                                    Alu.mult)
            if _dbg:
                nc.sync.dma_start(out=datt, in_=att[:])

            # ---- blockdiag(A) scatter ----
            nc.gpsimd.memset(A0[:], 0.0)
            nc.gpsimd.memset(A1[:], 0.0)
            for hh in range(HEADS):
                c0 = hh * CHD
                cs = slice(c0, c0 + CHD)
                srcb = att[:, cs]
                if c0 + CHD <= P0:
                    nc.sync.dma_start(out=A0[cs, cs], in_=srcb)
                elif c0 >= P0:
                    nc.sync.dma_start(out=A1[c0 - P0:c0 - P0 + CHD, cs], in_=srcb)
                else:
                    n0 = P0 - c0
                    nc.sync.dma_start(out=A0[c0:P0, cs], in_=srcb[0:n0, :])
                    nc.sync.dma_start(out=A1[0:CHD - n0, cs], in_=srcb[n0:CHD, :])

            # ================= v phase h1 pw =================
            pb = padp.tile([P0, 2 * PBSZ], f16, tag="pad")
            pad_zeros(pb, zero_top=False, zero_bot=True)
            pw_half(wv_sb, pb, 63, 128, -63)

            # ---- Wf = blockdiag(A)-contraction with WpT ----
            pwf0 = ps.tile([P0, 192], f32, tag="tap0")
            pwf1 = ps1.tile([P0, 192], f32, tag="tap1")
            nc.tensor.matmul(pwf0[:], A0[:, 0:P0], wp_sb[:, 0:192],
                             start=True, stop=False)
            nc.tensor.matmul(pwf0[:], A1[:, 0:P0], wp_sb[0:P1, 192:384],
                             start=False, stop=True)
            nc.tensor.matmul(pwf1[0:64], A0[:, P0:C], wp_sb[:, 0:192],
                             start=True, stop=False)
            nc.tensor.matmul(pwf1[0:64], A1[:, P0:C],
                             wp_sb[0:P1, 192:384],
                             start=False, stop=True)
            nc.vector.tensor_copy(wf_sb[:, 0:192], pwf0[:])
            nc.scalar.copy(wf_sb[0:64, 192:384], pwf1[0:64])
            if _dbg:
                nc.sync.dma_start(out=dwf[:], in_=wf_sb[:])

            def final_quarter(qt, px_base):
                for blk in range(8):
                    o = blk * SUB
                    po0 = ps.tile([P0, SUB], f32, tag="pw0")
                    po1 = ps.tile([P0, SUB], f32, tag="pw1")
                    nc.tensor.matmul(po0[:], wf_sb[:, 0:128],
                                     qt[:, o:o + SUB],
                                     start=True, stop=False)
                    nc.tensor.matmul(po0[:], wf_sb[0:64, 192:320],
                                     qt[0:64, 4096 + o:4096 + o + SUB],
                                     start=False, stop=True)
                    nc.tensor.matmul(po1[0:64], wf_sb[:, 128:192],
                                     qt[:, o:o + SUB],
                                     start=True, stop=False)
                    nc.tensor.matmul(po1[0:64], wf_sb[0:64, 320:384],
                                     qt[0:64, 4096 + o:4096 + o + SUB],
                                     start=False, stop=True)
                    ost = ostp.tile([P0, 1024], f16, tag="ost")
                    nc.vector.tensor_copy(ost[:, 0:SUB], po0[:])
                    nc.scalar.copy(ost[0:64, SUB:1024], po1[0:64])
                    px = px_base + o
                    nc.sync.dma_start(out=out[0:P0, px:px + SUB],
                                      in_=ost[:, 0:SUB])
                    nc.sync.dma_start(out=out[P0:C, px:px + SUB],
                                      in_=ost[0:64, SUB:1024])

            final_quarter(vq01[0], 0)
            final_quarter(vq01[1], 4096)

            # ================= v taps h1 + final =================
            vq23 = [vdwp.tile([P0, 2 * 4096], f16, tag="vdw",
                              name=f"vdw1_{i}") for i in range(2)]

            def v_sink1(blk, pt0, pt1):
                qt = vq23[blk // 8]
                o = (blk % 8) * SUB
                nc.vector.tensor_copy(qt[:, o:o + SUB], pt0[:])
                nc.scalar.copy(qt[0:64, 4096 + o:4096 + o + SUB], pt1[0:64])

            taps(pb, dvp_sb, 4, 16, 1, 128, v_sink1)
            final_quarter(vq23[0], 8192)
            final_quarter(vq23[1], 12288)

    nc.compile()
    return nc


def _host_inputs(x, w_qkv, w_dw, w_proj, temperature):
    f = np.float32
    h = np.float16
    W_q = w_qkv[0:C].astype(f)
    W_v = w_qkv[2 * C:3 * C].astype(f)
    wq_d = w_dw[0:C, 0].reshape(C, 9).astype(f)
    wv_d = w_dw[2 * C:3 * C, 0].reshape(C, 9).astype(f)

    def pack_T(Wm):
        o = np.zeros((P0, 384), f)
        WT = Wm.T.astype(f)
        o[:, 0:192] = WT[0:P0]
        o[64:128, 192:384] = WT[P0:C]
        return o

    def pack_diag(wd):
        o = np.zeros((P0, 576), f)
        for g in range(4):
            for t in range(9):
                np.fill_diagonal(o[32 * g:32 * g + 32, 32 * t:32 * t + 32],
                                 wd[32 * g:32 * g + 32, t])
        for j in range(2):
            for t in range(9):
                np.fill_diagonal(
                    o[64 + 32 * j:96 + 32 * j, 288 + 32 * t:288 + 32 * t + 32],
                    wd[128 + 32 * j:160 + 32 * j, t])
        return o

    wp_pack = np.zeros((P0, 384), f)
    WpT = w_proj.T.astype(f)
    wp_pack[:, 0:192] = WpT[0:P0]
    wp_pack[0:64, 192:384] = WpT[P0:C]

    tqv = np.sqrt(np.repeat(temperature.reshape(HEADS).astype(f), CHD)
                  ).reshape(C, 1)
    shared = {
        "wq": pack_T(W_q).astype(h), "wv": pack_T(W_v).astype(h),
        "wp": wp_pack,
        "dqp": pack_diag(wq_d).astype(h), "dvp": pack_diag(wv_d).astype(h),
        "tq": tqv,
    }
    maps = []
    for b in range(8):
        m = dict(shared)
        m["xb"] = np.ascontiguousarray(x[b].reshape(C, HW).astype(h))
        maps.append(m)
    return maps


def kernel(x, w_qkv, w_dw, w_proj, temperature, _trace=False, _iters=1):
    from concourse.bass_utils import run_bass_kernel_spmd
    if _iters not in _BUILT:
        _BUILT[_iters] = _build(_iters)
    nc = _BUILT[_iters]
    in_maps = _host_inputs(
        np.asarray(x), np.asarray(w_qkv), np.asarray(w_dw),
        np.asarray(w_proj), np.asarray(temperature))
    res = run_bass_kernel_spmd(nc, in_maps, list(range(8)), trace=_trace)
    outs = [res.results[i]["out"].reshape(C, H, W) for i in range(8)]
    y = np.stack(outs, axis=0).astype(np.float32)
    kernel.last_result = res
    return y
